# revision 49
# baseline (speedup 1.0000x reference)
"""DeepSeekMoE Trainium2 kernel (8 NeuronCores, expert-parallel).

Strategy
--------
Expert-parallel sharding: core c owns routed expert c plus 1/8 of the
tokens for the replicated shared expert.

The host performs only *integer* dispatch decisions (argmax top-2 of the
router logits): which tokens go to which expert, a balanced "shared
host" assignment giving every core exactly SH=512 of its own gathered
tokens to run through the replicated shared expert, and for each
gathered token a {+1,-1} one-hot *difference* mask dsel = onehot(own
expert) - onehot(partner expert).  Every floating-point value that
contributes to the output is computed on device:

  - each core re-computes the router logits for its gathered tokens from
    the bf16 x already resident for mm1 and derives the renormalized
    top-2 combine weight of its own expert as sigmoid(sum(logits*dsel))
    = sigmoid(l_own - l_other) = p_own/(p_own+p_other); the host masks
    pin the *selection* so bf16 logit noise only perturbs the weight
    value (~1e-3), never the expert choice,
  - mm1: hT[j] = gelu(w1.T-block @ xT) with the expert's w1 resident in
    SBUF (bf16), accumulated over 8 k-tiles in PSUM,
  - mm2: y[t,:] = (hT.T @ w2) * w_comb[t], w2 SBUF-resident (bf16),
  - the shared expert runs the same pipeline on a contiguous 512-token
    slice with sw1 streamed (each tile used once) and sw2 resident.

Phase order is chosen so the DMA ramp is hidden and the ACT table is
loaded exactly twice:

  shared mm1 (sw1 streamed on Pool-SWDGE paces compute from ~8us while
  w1/xgb/w2 land on the SP queue) -> expert mm1 -> expert mm2 with the
  router matmuls pumped between k-steps (Gelu era ends before the single
  Sigmoid table load) -> shared mm2 (sw2 resident).

Queues: SP-HWDGE carries xs0, w1, xgb, w2 then all output stores;
Pool-SWDGE carries sw1 tiles then resident sw2; Act-HWDGE carries only
the tiny router consts so the Scalar engine is otherwise free for
gelu/sigmoid/copy PSUM evictions (a blocked Act queue was the main
stall source in earlier revisions).  A dummy Gelu at program start
pre-warms the ACT table during the preamble.

Two empirically load-bearing scheduling devices (the Tile scheduler
reorders dependency-free DMAs, so emission order alone does not hold):

  - tile_wait_until marks on later-needed tensors (xgb c1/c2, w2, sw2)
    keep their transfers out of the bandwidth-critical first ~35us --
    the HW DMA rings round-robin descriptors, so any concurrent
    transfer delays the xs0/w1 completions the PE is waiting on;
  - SBUF allocation order separates the PE's moving-read tensors (xs0,
    xgb) from the DMA landing zones by the 53KB ht region, and the xs0
    k>=2 tiles are held back slightly: DMA writes landing near the
    tensor the PE is streaming halve the matmul rate (216ns -> 427ns
    slices).

All GEMMs run in bf16 (rel err ~4e-3, inside the 2e-2 gate); outputs
are stored bf16 to halve the drain traffic.  Host combine:
out[slice_c] = ys_c (shared), then out[idx_c] += yg_c per core --
index placement + the unavoidable unshard additions only.
"""

import sys

sys.path.insert(0, "/opt/trn_rl_repo")

from contextlib import ExitStack

import numpy as np
import ml_dtypes

import concourse.bass as bass  # noqa: F401  (engine types resolve through bacc)
import concourse.tile as tile
from concourse import bacc, mybir
from concourse.alu_op_type import AluOpType
from concourse.bass_utils import run_bass_kernel_spmd

F32 = mybir.dt.float32
BF16 = mybir.dt.bfloat16
BF = ml_dtypes.bfloat16
AF = mybir.ActivationFunctionType
X = mybir.AxisListType.X

D, H, E = 1024, 2048, 8
B, S = 2, 2048
T = B * S
NCORES = 8
SH = T // NCORES          # 512 shared-expert tokens per core
KD = D // 128             # 8 k-tiles over D
KH = H // 128             # 16 k-tiles over H
DEFAULT_CAP = 1152        # >= max per-expert token count (1091 for bench key)


def _chunks(cap):
    """Token chunks of <=512 (PSUM free-dim limit) covering [0, cap)."""
    out = []
    t0 = 0
    while t0 < cap:
        nt = min(512, cap - t0)
        out.append((t0, nt))
        t0 += nt
    return out


def build_program(cap: int, has_b1: bool, has_b2: bool, has_rb: bool):
    nc = bacc.Bacc("TRN2", debug=False)
    MTE = cap // 128          # expert mm2 m-tiles
    SME = SH // 128           # shared mm2 m-tiles

    xgt_b = nc.dram_tensor("xgt_b", [D, cap], BF16, kind="ExternalInput").ap()
    xs0 = nc.dram_tensor("xs0", [D, SH], BF16, kind="ExternalInput").ap()
    # rw / dsel arrive host-pre-permuted to [128, k*E] (partition-major) so
    # their DMAs are single wide-line transfers.
    rw = nc.dram_tensor("rw", [128, KD * E], BF16, kind="ExternalInput").ap()
    dsel = nc.dram_tensor("dsel", [128, MTE * E], F32, kind="ExternalInput").ap()
    rb = nc.dram_tensor("rb", [1, E], F32, kind="ExternalInput").ap()
    w1 = nc.dram_tensor("w1", [D, H], BF16, kind="ExternalInput").ap()
    w2 = nc.dram_tensor("w2", [H, D], BF16, kind="ExternalInput").ap()
    sw1 = nc.dram_tensor("sw1", [D, H], BF16, kind="ExternalInput").ap()
    sw2 = nc.dram_tensor("sw2", [H, D], BF16, kind="ExternalInput").ap()
    b1 = nc.dram_tensor("b1", [1, H], BF16, kind="ExternalInput").ap()
    b2 = nc.dram_tensor("b2", [1, D], BF16, kind="ExternalInput").ap()
    sb1 = nc.dram_tensor("sb1", [1, H], BF16, kind="ExternalInput").ap()
    sb2 = nc.dram_tensor("sb2", [1, D], BF16, kind="ExternalInput").ap()
    yg = nc.dram_tensor("yg", [cap, D], BF16, kind="ExternalOutput").ap()
    ys = nc.dram_tensor("ys", [SH, D], BF16, kind="ExternalOutput").ap()

    with tile.TileContext(nc) as tc, ExitStack() as ctx:
        const = ctx.enter_context(tc.tile_pool(name="const", bufs=1))
        big = ctx.enter_context(tc.tile_pool(name="big", bufs=1))
        rpool = ctx.enter_context(tc.tile_pool(name="rpool", bufs=2))
        sw1p = ctx.enter_context(tc.tile_pool(name="sw1p", bufs=16))
        ost = ctx.enter_context(tc.tile_pool(name="ost", bufs=8))
        psp = ctx.enter_context(tc.tile_pool(name="psp", bufs=7, space="PSUM"))
        lpp = ctx.enter_context(tc.tile_pool(name="lpp", bufs=1, space="PSUM"))

        # ---- ACT table pre-warm: the Gelu table load (~1.3us) runs during
        # the framework preamble instead of stalling the first eviction ----
        warm_in = const.tile([1, 32], F32, tag="warm_in")
        nc.vector.memset(warm_in, 0.0)
        warm_out = const.tile([1, 32], F32, tag="warm_out")
        nc.scalar.activation(warm_out, warm_in[:], AF.Gelu)

        # ---- rare bias consts (absent on the bench path) ----
        if has_rb:
            ones_f = const.tile([1, 128], F32, tag="ones_f")
            nc.vector.memset(ones_f, 1.0)
            rb_sb = const.tile([1, E], F32, tag="rb")
            nc.scalar.dma_start(out=rb_sb, in_=rb)
        if has_b1:
            ones_tf = const.tile([1, 512], F32, tag="ones_tf")
            nc.vector.memset(ones_tf, 1.0)
            ones_tok = const.tile([1, 512], BF16, tag="ones_tok")
            nc.vector.tensor_copy(ones_tok, ones_tf[:])
            b1row = const.tile([1, H], BF16, tag="b1row")
            nc.scalar.dma_start(out=b1row, in_=b1)
            sb1row = const.tile([1, H], BF16, tag="sb1row")
            nc.scalar.dma_start(out=sb1row, in_=sb1)
        if has_b2:
            ones_mf = const.tile([1, 128], F32, tag="ones_mf")
            nc.vector.memset(ones_mf, 1.0)
            onesm_b = const.tile([1, 128], BF16, tag="onesm_b")
            nc.vector.tensor_copy(onesm_b, ones_mf[:])
            b2row = const.tile([1, D], BF16, tag="b2row")
            nc.scalar.dma_start(out=b2row, in_=b2)
            sb2row = const.tile([1, D], BF16, tag="sb2row")
            nc.scalar.dma_start(out=sb2row, in_=sb2)

        # ---- resident tensors, all on the SP queue in need order:
        # xs0 (shared mm1, from ~8us) -> w1, xgb (expert mm1, from ~36us)
        # -> w2 (expert mm2, from ~97us).  Output stores are queued behind
        # these later; SP is idle by the time the first eviction lands. ----
        # Allocation order inside `big` separates the PE's moving-read
        # tensors (xs0, xgb) from the weight landing zones by the 53KB ht
        # region: concurrent DMA writes near the tensor the PE is streaming
        # halve the matmul rate (observed 216ns -> 427ns slices).
        xs0_sb = big.tile([128, KD, SH], BF16, tag="xs0")
        xgb_sb = big.tile([128, KD, cap], BF16, tag="xgb")
        ht = big.tile([128, KH, cap + SH], BF16, tag="ht")
        w1_sb = big.tile([128, KD, H], BF16, tag="w1res")
        w2_sb = big.tile([128, KH, D], BF16, tag="w2res")
        sw2_sb = big.tile([128, KH, D], BF16, tag="sw2res")

        # xs0 = the core's SH shared-hosted tokens (also rows 0..SH of the
        # gather, shipped twice).  Its k-tiles land first and pace the
        # shared mm1 ramp; nothing else is critical before ~virtual 25us.
        # all of xs0 lands first and unpinned: a wait mark here pushes the
        # tail k-tiles behind the unpinned w1 burst on the SP queue, and the
        # PE's k-step then idles ~3.5us for them (ring-sem traced)
        xs0_r = xs0.rearrange("(k p) t -> p k t", p=128)
        for k in range(KD):
            nc.sync.dma_start(out=xs0_sb[:, k, :], in_=xs0_r[:, k, :])
        # w1 in four j-quarters: expert mm1's q-groups consume columns
        # progressively (quarter qq first needed ~virtual 31 + 6.8*qq), so
        # only 1MB of w1 competes inside the critical startup window
        w1_r = w1.rearrange("(k p) h -> p k h", p=128)
        for qq in range(4):
            with tc.tile_wait_until(0.008 * qq, enable=(qq > 0)):
                for k in range(KD):
                    nc.sync.dma_start(
                        out=w1_sb[:, k, qq * 512 : (qq + 1) * 512],
                        in_=w1_r[:, k, qq * 512 : (qq + 1) * 512])
        # Later-needed tensors carry tile_wait_until marks so the scheduler
        # cannot hoist their transfers into the bandwidth-critical first
        # ~35us (HW DMA rings round-robin: every concurrent transfer delays
        # the critical xs0/w1 completions the PE is waiting on).
        xgb_r = xgt_b.rearrange("(k p) t -> p k t", p=128)
        chunks = _chunks(cap)
        for ci, (t0, nt) in enumerate(chunks):
            with tc.tile_wait_until(0.008 if ci == 0 else 0.020 + 0.005 * ci):
                for k in range(KD):
                    nc.sync.dma_start(out=xgb_sb[:, k, t0 : t0 + nt],
                                      in_=xgb_r[:, k, t0 : t0 + nt])
        w2_r = w2.rearrange("(k p) d -> p k d", p=128)
        with tc.tile_wait_until(0.040):
            for k in range(KH):
                nc.sync.dma_start(out=w2_sb[:, k, :], in_=w2_r[:, k, :])
        sw2_r = sw2.rearrange("(k p) d -> p k d", p=128)

        # ---- shared mm1 first: sw1 streamed on the Pool queue paces the
        # PE from ~8us while the big expert tensors land on SP ----
        for q in range(4):
            phs = []
            for mh in range(4):
                j = q * 4 + mh
                ph = psp.tile([128, SH], F32, tag="ps", name=f"sph{q}_{mh}")
                phs.append(ph)
                if has_b1:
                    nc.tensor.matmul(
                        ph, sb1row[:, j * 128 : (j + 1) * 128],
                        ones_tok[:, :SH], start=True, stop=False)
            for k in range(KD):
                swt = sw1p.tile([128, 512], BF16, tag="sw1t", name=f"sw1_{q}_{k}")
                nc.gpsimd.dma_start(
                    out=swt,
                    in_=sw1[k * 128 : (k + 1) * 128, q * 512 : (q + 1) * 512])
                for mh in range(4):
                    nc.tensor.matmul(
                        phs[mh],
                        swt[:, mh * 128 : (mh + 1) * 128],
                        xs0_sb[:, k, :],
                        start=(k == 0 and not has_b1),
                        stop=(k == KD - 1))
            for mh in range(4):
                j = q * 4 + mh
                nc.scalar.activation(ht[:, j, cap : cap + SH], phs[mh][:], AF.Gelu)

        # router consts on the Act queue (needed ~100us; keep clear of the
        # startup window)
        with tc.tile_wait_until(0.010):
            rw_sb = const.tile([128, KD, E], BF16, tag="rw")
            nc.scalar.dma_start(out=rw_sb, in_=rw.rearrange("p (k e) -> p k e", k=KD))
            dsel_sb = const.tile([128, MTE, E], F32, tag="dsel")
            nc.scalar.dma_start(out=dsel_sb,
                                in_=dsel.rearrange("p (m e) -> p m e", m=MTE))

        # sw2 resident on Pool behind the sw1 stream (needed ~160us)
        with tc.tile_wait_until(0.050):
            for k in range(KH):
                nc.gpsimd.dma_start(out=sw2_sb[:, k, :], in_=sw2_r[:, k, :])

        # ---- expert mm1: ht[j] = gelu(w1.T-block @ xT), w1 resident ----
        for ci, (t0, nt) in enumerate(chunks):
            for q in range(8):
                phs = []
                for mh in range(2):
                    j = q * 2 + mh
                    ph = psp.tile([128, nt], F32, tag="ps", name=f"ph{t0}_{q}_{mh}")
                    phs.append(ph)
                    if has_b1:
                        nc.tensor.matmul(
                            ph, b1row[:, j * 128 : (j + 1) * 128],
                            ones_tok[:, :nt], start=True, stop=False)
                for k in range(KD):
                    for mh in range(2):
                        j = q * 2 + mh
                        nc.tensor.matmul(
                            phs[mh],
                            w1_sb[:, k, j * 128 : (j + 1) * 128],
                            xgb_sb[:, k, t0 : t0 + nt],
                            start=(k == 0 and not has_b1),
                            stop=(k == KD - 1))
                for mh in range(2):
                    j = q * 2 + mh
                    nc.scalar.activation(ht[:, j, t0 : t0 + nt], phs[mh][:], AF.Gelu)

        # ---- router: bf16 logits from the resident xgb; host dsel masks
        # fold selection + combine into sigmoid(sum(l*dsel)).  The matmuls
        # are pumped one per mm2 k-step so their weight loads hide. ----
        lp_all = lpp.tile([128, MTE, E], F32, tag="lp")
        wv = [const.tile([128, 1], F32, tag=f"wv{mt}", name=f"wv{mt}")
              for mt in range(MTE)]

        def router_steps():
            for mt in range(MTE):
                ts = slice(mt * 128, (mt + 1) * 128)
                for k in range(KD):
                    nc.tensor.matmul(
                        lp_all[:, mt, :],
                        xgb_sb[:, k, ts],
                        rw_sb[:, k, :],
                        start=(k == 0),
                        stop=(k == KD - 1 and not has_rb),
                        skip_group_check=True,
                    )
                    if k < KD - 1:
                        yield 0
                if has_rb:
                    nc.tensor.matmul(lp_all[:, mt, :], ones_f[:], rb_sb[:],
                                     start=False, stop=True, skip_group_check=True)
                g = rpool.tile([128, E], F32, tag="g", name=f"g{mt}")
                nc.vector.tensor_tensor(g, lp_all[:, mt, :], dsel_sb[:, mt, :],
                                        op=AluOpType.mult)
                gap = rpool.tile([128, 1], F32, tag="gap", name=f"gap{mt}")
                nc.vector.reduce_sum(gap, g[:], axis=X)
                nc.scalar.activation(wv[mt], gap[:], AF.Sigmoid)
                yield 0

        router_it = router_steps()

        def pump_router():
            try:
                next(router_it)
            except StopIteration:
                pass

        yg_r = yg.rearrange("(m p) d -> p m d", p=128)
        ys_r = ys.rearrange("(m p) d -> p m d", p=128)

        def mm2_group(mt, ht_col0, w2t, seed, out_r, scale, gname,
                      interleave_router=False, last=False):
            """One mm2 m-tile: 2 n-halves x full k loop, bf16 eviction."""
            pos = []
            for n in range(2):
                po = psp.tile([128, 512], F32, tag="ps", name=f"po{gname}_{n}")
                pos.append(po)
                if seed is not None:
                    nc.tensor.matmul(
                        po, onesm_b[:], seed[:, n * 512 : (n + 1) * 512],
                        start=True, stop=False)
            for k in range(KH):
                for n in range(2):
                    nc.tensor.matmul(
                        pos[n],
                        ht[:, k, ht_col0 : ht_col0 + 128],
                        w2t[:, k, n * 512 : (n + 1) * 512],
                        start=(k == 0 and seed is None),
                        stop=(k == KH - 1))
                if interleave_router:
                    pump_router()
            for n in range(2):
                og = ost.tile([128, 512], BF16, tag="og", name=f"og{gname}_{n}")
                # split the two n-half evictions across DVE and ACT so each
                # group's drain runs on two engines in parallel
                if scale:
                    if n == 0:
                        nc.vector.tensor_scalar(
                            og, pos[n][:], wv[mt][:], None, op0=AluOpType.mult)
                    else:
                        nc.scalar.activation(
                            og, pos[n][:], AF.Copy, scale=wv[mt][:])
                else:
                    if n == 0:
                        nc.vector.tensor_copy(og, pos[n][:])
                    else:
                        nc.scalar.activation(og, pos[n][:], AF.Copy)
                # for the final group issue the two stores on different
                # queues so the terminal drain is one store, not two
                oeng = nc.scalar if (last and n == 1) else nc.sync
                oeng.dma_start(
                    out=out_r[:, mt, n * 512 : (n + 1) * 512], in_=og)

        # ---- expert mm2 (w2 resident; router pumped between k-steps) ----
        eb2seed = b2row if has_b2 else None
        sb2seed = sb2row if has_b2 else None
        for mt in range(MTE):
            mm2_group(mt, mt * 128, w2_sb, eb2seed, yg_r, True, f"e{mt}",
                      interleave_router=True)

        # ---- shared mm2 (sw2 resident) ----
        for mt in range(SME):
            mm2_group(mt, cap + mt * 128, sw2_sb, sb2seed, ys_r, False, f"s{mt}",
                      last=(mt == SME - 1))

    nc.compile()
    return nc


_programs: dict = {}
LAST_RESULTS = None


def _get_program(key):
    if key not in _programs:
        _programs[key] = build_program(*key)
    return _programs[key]


def kernel(x, router_w, router_b, sw1, sb1, sw2, sb2, ew1, eb1, ew2, eb2):
    x = np.asarray(x, dtype=np.float32)
    flat = np.ascontiguousarray(x.reshape(T, D))
    rw_in = np.ascontiguousarray(np.asarray(router_w, np.float32))
    rb_in = np.asarray(router_b, np.float32).reshape(1, E)

    # Integer dispatch decisions only: which two experts each token visits.
    logits = flat @ rw_in + rb_in
    top1 = np.argmax(logits, axis=1)
    l2m = np.array(logits)
    l2m[np.arange(T), top1] = -np.inf
    top2 = np.argmax(l2m, axis=1)
    idxs = [np.flatnonzero((top1 == e) | (top2 == e)) for e in range(E)]

    # Assign each token a "shared host" core among its top-2 experts with
    # exactly SH tokens per core, so the shared-expert input is a subset of
    # the core's gathered tokens (saves a separate xsb shard + its DMA).
    # Integer decisions only.  Fallback: excess tokens append to any
    # under-loaded core as pseudo-gathered rows (their yg is discarded).
    host = top1.copy()
    load = np.bincount(host, minlength=E)
    for _ in range(4 * E):
        over = np.flatnonzero(load > SH)
        if len(over) == 0:
            break
        moved = False
        for c in over:
            for t in np.flatnonzero(host == c):
                alt = top2[t] if top1[t] == c else top1[t]
                if load[alt] < SH:
                    host[t] = alt
                    load[c] -= 1
                    load[alt] += 1
                    moved = True
                    if load[c] <= SH:
                        break
        if not moved:
            break
    for c in np.flatnonzero(load > SH):  # last resort: append anywhere
        for t in np.flatnonzero(host == c):
            if load[c] <= SH:
                break
            u = int(np.argmin(load))
            host[t] = u
            load[c] -= 1
            load[u] += 1

    gathers = []
    for c in range(NCORES):
        sh_c = np.flatnonzero(host == c)
        rest = idxs[c][host[idxs[c]] != c]
        gathers.append(np.concatenate([sh_c, rest]))
    maxn = max(len(g) for g in gathers)
    cap = max(DEFAULT_CAP, ((maxn + 127) // 128) * 128)
    MTE = cap // 128

    has_b1 = bool(np.any(sb1)) or bool(np.any(eb1))
    has_b2 = bool(np.any(sb2)) or bool(np.any(eb2))
    has_rb = bool(np.any(router_b))
    nc = _get_program((cap, has_b1, has_b2, has_rb))

    # rw / dsel pre-permuted to [128, k*E] (partition-major) wide-line DMAs
    rw_perm = np.ascontiguousarray(
        rw_in.reshape(KD, 128, E).transpose(1, 0, 2).reshape(128, KD * E)
    ).astype(BF)

    sw1b = np.ascontiguousarray(np.asarray(sw1)).astype(BF)
    sw2b = np.ascontiguousarray(np.asarray(sw2)).astype(BF)
    sb1b = np.asarray(sb1).reshape(1, H).astype(BF)
    sb2b = np.asarray(sb2).reshape(1, D).astype(BF)
    ew1a = np.asarray(ew1)
    ew2a = np.asarray(ew2)
    eb1a = np.asarray(eb1)
    eb2a = np.asarray(eb2)

    in_maps = []
    real_rows = []
    for c in range(NCORES):
        g = gathers[c]
        n = len(g)
        xg = np.zeros((cap, D), np.float32)
        xg[:n] = flat[g]
        xgt_b = np.ascontiguousarray(xg.T).astype(BF)
        # dsel: +1 at own expert, -1 at the partner expert (integer
        # dispatch info); zero rows for padding and for appended
        # shared-only pseudo rows -> their yg contribution is discarded.
        is_real = (top1[g] == c) | (top2[g] == c)
        rr = np.flatnonzero(is_real)
        real_rows.append(rr)
        other = np.where(top1[g[rr]] == c, top2[g[rr]], top1[g[rr]])
        dsel_c = np.zeros((cap, E), np.float32)
        dsel_c[rr, c] = 1.0
        dsel_c[rr, other] = -1.0
        dsel_perm = np.ascontiguousarray(
            dsel_c.reshape(MTE, 128, E).transpose(1, 0, 2).reshape(128, MTE * E))
        in_maps.append({
            "xgt_b": xgt_b,
            "xs0": np.ascontiguousarray(flat[g[:SH]].T).astype(BF),
            "rw": rw_perm,
            "dsel": dsel_perm,
            "rb": rb_in,
            "w1": np.ascontiguousarray(ew1a[c]).astype(BF),
            "w2": np.ascontiguousarray(ew2a[c]).astype(BF),
            "sw1": sw1b,
            "sw2": sw2b,
            "b1": np.asarray(eb1a[c]).reshape(1, H).astype(BF),
            "b2": np.asarray(eb2a[c]).reshape(1, D).astype(BF),
            "sb1": sb1b,
            "sb2": sb2b,
        })

    res = None
    for attempt in range(5):
        try:
            res = run_bass_kernel_spmd(nc, in_maps, core_ids=list(range(NCORES)))
            break
        except Exception:
            if attempt == 4:
                raise
            import time as _time
            _time.sleep(25)  # wedged-device windows recover after ~1-2 min
    global LAST_RESULTS
    LAST_RESULTS = res

    y = np.zeros((T, D), np.float32)
    for c in range(NCORES):
        g = gathers[c]
        y[g[:SH]] += res.results[c]["ys"].astype(np.float32)
        rr = real_rows[c]
        y[g[rr]] += res.results[c]["yg"][rr].astype(np.float32)
    return y.reshape(B, S, D)


# revision 50
# speedup vs baseline: 1.0107x; 1.0107x over previous
"""DeepSeekMoE Trainium2 kernel (8 NeuronCores, expert-parallel).

Strategy
--------
Expert-parallel sharding: core c owns routed expert c plus 1/8 of the
tokens for the replicated shared expert.

The host performs only *integer* dispatch decisions (argmax top-2 of the
router logits): which tokens go to which expert, a balanced "shared
host" assignment giving every core exactly SH=512 of its own gathered
tokens to run through the replicated shared expert, and for each
gathered token a {+1,-1} one-hot *difference* mask dsel = onehot(own
expert) - onehot(partner expert).  Every floating-point value that
contributes to the output is computed on device:

  - each core re-computes the router logits for its gathered tokens from
    the bf16 x already resident for mm1 and derives the renormalized
    top-2 combine weight of its own expert as sigmoid(sum(logits*dsel))
    = sigmoid(l_own - l_other) = p_own/(p_own+p_other); the host masks
    pin the *selection* so bf16 logit noise only perturbs the weight
    value (~1e-3), never the expert choice,
  - mm1: hT[j] = gelu(w1.T-block @ xT) with the expert's w1 resident in
    SBUF (bf16), accumulated over 8 k-tiles in PSUM,
  - mm2: y[t,:] = (hT.T @ w2) * w_comb[t], w2 SBUF-resident (bf16),
  - the shared expert runs the same pipeline on a contiguous 512-token
    slice with sw1 streamed (each tile used once) and sw2 resident.

Phase order is chosen so the DMA ramp is hidden and the ACT table is
loaded exactly twice:

  shared mm1 (sw1 streamed on Pool-SWDGE paces compute from ~8us while
  w1/xgb/w2 land on the SP queue) -> expert mm1 -> expert mm2 with the
  router matmuls pumped between k-steps (Gelu era ends before the single
  Sigmoid table load) -> shared mm2 (sw2 resident).

Queues: SP-HWDGE carries xs0, w1, xgb, w2 then all output stores;
Pool-SWDGE carries sw1 tiles then resident sw2; Act-HWDGE carries only
the tiny router consts so the Scalar engine is otherwise free for
gelu/sigmoid/copy PSUM evictions (a blocked Act queue was the main
stall source in earlier revisions).  A dummy Gelu at program start
pre-warms the ACT table during the preamble.

Two empirically load-bearing scheduling devices (the Tile scheduler
reorders dependency-free DMAs, so emission order alone does not hold):

  - tile_wait_until marks on later-needed tensors (xgb c1/c2, w2, sw2)
    keep their transfers out of the bandwidth-critical first ~35us --
    the HW DMA rings round-robin descriptors, so any concurrent
    transfer delays the xs0/w1 completions the PE is waiting on;
  - SBUF allocation order separates the PE's moving-read tensors (xs0,
    xgb) from the DMA landing zones by the 53KB ht region, and the xs0
    k>=2 tiles are held back slightly: DMA writes landing near the
    tensor the PE is streaming halve the matmul rate (216ns -> 427ns
    slices).

All GEMMs run in bf16 (rel err ~4e-3, inside the 2e-2 gate); outputs
are stored bf16 to halve the drain traffic.  Host combine:
out[slice_c] = ys_c (shared), then out[idx_c] += yg_c per core --
index placement + the unavoidable unshard additions only.
"""

import sys

sys.path.insert(0, "/opt/trn_rl_repo")

from contextlib import ExitStack

import numpy as np
import ml_dtypes

import concourse.bass as bass  # noqa: F401  (engine types resolve through bacc)
import concourse.tile as tile
from concourse import bacc, mybir
from concourse.alu_op_type import AluOpType
from concourse.bass_utils import run_bass_kernel_spmd

F32 = mybir.dt.float32
BF16 = mybir.dt.bfloat16
BF = ml_dtypes.bfloat16
AF = mybir.ActivationFunctionType
X = mybir.AxisListType.X

D, H, E = 1024, 2048, 8
B, S = 2, 2048
T = B * S
NCORES = 8
SH = T // NCORES          # 512 shared-expert tokens per core
KD = D // 128             # 8 k-tiles over D
KH = H // 128             # 16 k-tiles over H
DEFAULT_CAP = 1152        # >= max per-expert token count (1091 for bench key)


def _chunks(cap):
    """Token chunks of <=512 (PSUM free-dim limit) covering [0, cap)."""
    out = []
    t0 = 0
    while t0 < cap:
        nt = min(512, cap - t0)
        out.append((t0, nt))
        t0 += nt
    return out


def build_program(cap: int, has_b1: bool, has_b2: bool, has_rb: bool):
    nc = bacc.Bacc("TRN2", debug=False)
    MTE = cap // 128          # expert mm2 m-tiles
    SME = SH // 128           # shared mm2 m-tiles

    xgt_b = nc.dram_tensor("xgt_b", [D, cap], BF16, kind="ExternalInput").ap()
    xs0 = nc.dram_tensor("xs0", [D, SH], BF16, kind="ExternalInput").ap()
    # rw / dsel arrive host-pre-permuted to [128, k*E] (partition-major) so
    # their DMAs are single wide-line transfers.
    rw = nc.dram_tensor("rw", [128, KD * E], BF16, kind="ExternalInput").ap()
    dsel = nc.dram_tensor("dsel", [128, MTE * E], F32, kind="ExternalInput").ap()
    rb = nc.dram_tensor("rb", [1, E], F32, kind="ExternalInput").ap()
    w1 = nc.dram_tensor("w1", [D, H], BF16, kind="ExternalInput").ap()
    w2 = nc.dram_tensor("w2", [H, D], BF16, kind="ExternalInput").ap()
    sw1 = nc.dram_tensor("sw1", [D, H], BF16, kind="ExternalInput").ap()
    sw2 = nc.dram_tensor("sw2", [H, D], BF16, kind="ExternalInput").ap()
    b1 = nc.dram_tensor("b1", [1, H], BF16, kind="ExternalInput").ap()
    b2 = nc.dram_tensor("b2", [1, D], BF16, kind="ExternalInput").ap()
    sb1 = nc.dram_tensor("sb1", [1, H], BF16, kind="ExternalInput").ap()
    sb2 = nc.dram_tensor("sb2", [1, D], BF16, kind="ExternalInput").ap()
    yg = nc.dram_tensor("yg", [cap, D], BF16, kind="ExternalOutput").ap()
    ys = nc.dram_tensor("ys", [SH, D], BF16, kind="ExternalOutput").ap()

    with tile.TileContext(nc) as tc, ExitStack() as ctx:
        const = ctx.enter_context(tc.tile_pool(name="const", bufs=1))
        big = ctx.enter_context(tc.tile_pool(name="big", bufs=1))
        rpool = ctx.enter_context(tc.tile_pool(name="rpool", bufs=2))
        sw1p = ctx.enter_context(tc.tile_pool(name="sw1p", bufs=16))
        ost = ctx.enter_context(tc.tile_pool(name="ost", bufs=8))
        psp = ctx.enter_context(tc.tile_pool(name="psp", bufs=7, space="PSUM"))
        lpp = ctx.enter_context(tc.tile_pool(name="lpp", bufs=1, space="PSUM"))

        # ---- ACT table pre-warm: the Gelu table load (~1.3us) runs during
        # the framework preamble instead of stalling the first eviction ----
        warm_in = const.tile([1, 32], F32, tag="warm_in")
        nc.vector.memset(warm_in, 0.0)
        warm_out = const.tile([1, 32], F32, tag="warm_out")
        nc.scalar.activation(warm_out, warm_in[:], AF.Gelu)

        # ---- rare bias consts (absent on the bench path) ----
        if has_rb:
            ones_f = const.tile([1, 128], F32, tag="ones_f")
            nc.vector.memset(ones_f, 1.0)
            rb_sb = const.tile([1, E], F32, tag="rb")
            nc.scalar.dma_start(out=rb_sb, in_=rb)
        if has_b1:
            ones_tf = const.tile([1, 512], F32, tag="ones_tf")
            nc.vector.memset(ones_tf, 1.0)
            ones_tok = const.tile([1, 512], BF16, tag="ones_tok")
            nc.vector.tensor_copy(ones_tok, ones_tf[:])
            b1row = const.tile([1, H], BF16, tag="b1row")
            nc.scalar.dma_start(out=b1row, in_=b1)
            sb1row = const.tile([1, H], BF16, tag="sb1row")
            nc.scalar.dma_start(out=sb1row, in_=sb1)
        if has_b2:
            ones_mf = const.tile([1, 128], F32, tag="ones_mf")
            nc.vector.memset(ones_mf, 1.0)
            onesm_b = const.tile([1, 128], BF16, tag="onesm_b")
            nc.vector.tensor_copy(onesm_b, ones_mf[:])
            b2row = const.tile([1, D], BF16, tag="b2row")
            nc.scalar.dma_start(out=b2row, in_=b2)
            sb2row = const.tile([1, D], BF16, tag="sb2row")
            nc.scalar.dma_start(out=sb2row, in_=sb2)

        # ---- resident tensors, all on the SP queue in need order:
        # xs0 (shared mm1, from ~8us) -> w1, xgb (expert mm1, from ~36us)
        # -> w2 (expert mm2, from ~97us).  Output stores are queued behind
        # these later; SP is idle by the time the first eviction lands. ----
        # Allocation order inside `big` separates the PE's moving-read
        # tensors (xs0, xgb) from the weight landing zones by the 53KB ht
        # region: concurrent DMA writes near the tensor the PE is streaming
        # halve the matmul rate (observed 216ns -> 427ns slices).
        xs0_sb = big.tile([128, KD, SH], BF16, tag="xs0")
        xgb_sb = big.tile([128, KD, cap], BF16, tag="xgb")
        ht = big.tile([128, KH, cap + SH], BF16, tag="ht")
        w1_sb = big.tile([128, KD, H], BF16, tag="w1res")
        w2_sb = big.tile([128, KH, D], BF16, tag="w2res")
        sw2_sb = big.tile([128, KH, D], BF16, tag="sw2res")

        # xs0 = the core's SH shared-hosted tokens (also rows 0..SH of the
        # gather, shipped twice).  Its k-tiles land first and pace the
        # shared mm1 ramp; nothing else is critical before ~virtual 25us.
        # stagger the later xs0 k-tiles: DMA writes landing in the tensor
        # the PE is streaming halve its rate (A/B-measured better than
        # letting the whole burst land up front, despite a ~3us k2 wait)
        xs0_r = xs0.rearrange("(k p) t -> p k t", p=128)
        for k in range(KD):
            with tc.tile_wait_until(0.003, enable=(k >= 2)):
                nc.sync.dma_start(out=xs0_sb[:, k, :], in_=xs0_r[:, k, :])
        # w1 in four j-quarters: expert mm1's q-groups consume columns
        # progressively (quarter qq first needed ~virtual 31 + 6.8*qq), so
        # only 1MB of w1 competes inside the critical startup window
        w1_r = w1.rearrange("(k p) h -> p k h", p=128)
        for qq in range(4):
            with tc.tile_wait_until(0.008 * qq, enable=(qq > 0)):
                for k in range(KD):
                    nc.sync.dma_start(
                        out=w1_sb[:, k, qq * 512 : (qq + 1) * 512],
                        in_=w1_r[:, k, qq * 512 : (qq + 1) * 512])
        # Later-needed tensors carry tile_wait_until marks so the scheduler
        # cannot hoist their transfers into the bandwidth-critical first
        # ~35us (HW DMA rings round-robin: every concurrent transfer delays
        # the critical xs0/w1 completions the PE is waiting on).
        xgb_r = xgt_b.rearrange("(k p) t -> p k t", p=128)
        chunks = _chunks(cap)
        for ci, (t0, nt) in enumerate(chunks):
            with tc.tile_wait_until(0.008 if ci == 0 else 0.020 + 0.005 * ci):
                for k in range(KD):
                    nc.sync.dma_start(out=xgb_sb[:, k, t0 : t0 + nt],
                                      in_=xgb_r[:, k, t0 : t0 + nt])
        w2_r = w2.rearrange("(k p) d -> p k d", p=128)
        with tc.tile_wait_until(0.040):
            for k in range(KH):
                nc.sync.dma_start(out=w2_sb[:, k, :], in_=w2_r[:, k, :])
        sw2_r = sw2.rearrange("(k p) d -> p k d", p=128)

        # ---- shared mm1 first: sw1 streamed on the Pool queue paces the
        # PE from ~8us while the big expert tensors land on SP ----
        for q in range(4):
            phs = []
            for mh in range(4):
                j = q * 4 + mh
                ph = psp.tile([128, SH], F32, tag="ps", name=f"sph{q}_{mh}")
                phs.append(ph)
                if has_b1:
                    nc.tensor.matmul(
                        ph, sb1row[:, j * 128 : (j + 1) * 128],
                        ones_tok[:, :SH], start=True, stop=False)
            for k in range(KD):
                swt = sw1p.tile([128, 512], BF16, tag="sw1t", name=f"sw1_{q}_{k}")
                nc.gpsimd.dma_start(
                    out=swt,
                    in_=sw1[k * 128 : (k + 1) * 128, q * 512 : (q + 1) * 512])
                for mh in range(4):
                    nc.tensor.matmul(
                        phs[mh],
                        swt[:, mh * 128 : (mh + 1) * 128],
                        xs0_sb[:, k, :],
                        start=(k == 0 and not has_b1),
                        stop=(k == KD - 1))
            for mh in range(4):
                j = q * 4 + mh
                nc.scalar.activation(ht[:, j, cap : cap + SH], phs[mh][:], AF.Gelu)

        # router consts on the Act queue (needed ~100us; keep clear of the
        # startup window)
        with tc.tile_wait_until(0.010):
            rw_sb = const.tile([128, KD, E], BF16, tag="rw")
            nc.scalar.dma_start(out=rw_sb, in_=rw.rearrange("p (k e) -> p k e", k=KD))
            dsel_sb = const.tile([128, MTE, E], F32, tag="dsel")
            nc.scalar.dma_start(out=dsel_sb,
                                in_=dsel.rearrange("p (m e) -> p m e", m=MTE))

        # sw2 resident on Pool behind the sw1 stream (needed ~160us)
        with tc.tile_wait_until(0.050):
            for k in range(KH):
                nc.gpsimd.dma_start(out=sw2_sb[:, k, :], in_=sw2_r[:, k, :])

        # ---- expert mm1: ht[j] = gelu(w1.T-block @ xT), w1 resident ----
        for ci, (t0, nt) in enumerate(chunks):
            for q in range(8):
                phs = []
                for mh in range(2):
                    j = q * 2 + mh
                    ph = psp.tile([128, nt], F32, tag="ps", name=f"ph{t0}_{q}_{mh}")
                    phs.append(ph)
                    if has_b1:
                        nc.tensor.matmul(
                            ph, b1row[:, j * 128 : (j + 1) * 128],
                            ones_tok[:, :nt], start=True, stop=False)
                for k in range(KD):
                    for mh in range(2):
                        j = q * 2 + mh
                        nc.tensor.matmul(
                            phs[mh],
                            w1_sb[:, k, j * 128 : (j + 1) * 128],
                            xgb_sb[:, k, t0 : t0 + nt],
                            start=(k == 0 and not has_b1),
                            stop=(k == KD - 1))
                for mh in range(2):
                    j = q * 2 + mh
                    nc.scalar.activation(ht[:, j, t0 : t0 + nt], phs[mh][:], AF.Gelu)

        # ---- router: bf16 logits from the resident xgb; host dsel masks
        # fold selection + combine into sigmoid(sum(l*dsel)).  The matmuls
        # are pumped one per mm2 k-step so their weight loads hide. ----
        lp_all = lpp.tile([128, MTE, E], F32, tag="lp")
        wv = [const.tile([128, 1], F32, tag=f"wv{mt}", name=f"wv{mt}")
              for mt in range(MTE)]

        def router_steps():
            for mt in range(MTE):
                ts = slice(mt * 128, (mt + 1) * 128)
                for k in range(KD):
                    nc.tensor.matmul(
                        lp_all[:, mt, :],
                        xgb_sb[:, k, ts],
                        rw_sb[:, k, :],
                        start=(k == 0),
                        stop=(k == KD - 1 and not has_rb),
                        skip_group_check=True,
                    )
                    if k < KD - 1:
                        yield 0
                if has_rb:
                    nc.tensor.matmul(lp_all[:, mt, :], ones_f[:], rb_sb[:],
                                     start=False, stop=True, skip_group_check=True)
                g = rpool.tile([128, E], F32, tag="g", name=f"g{mt}")
                nc.vector.tensor_tensor(g, lp_all[:, mt, :], dsel_sb[:, mt, :],
                                        op=AluOpType.mult)
                gap = rpool.tile([128, 1], F32, tag="gap", name=f"gap{mt}")
                nc.vector.reduce_sum(gap, g[:], axis=X)
                nc.scalar.activation(wv[mt], gap[:], AF.Sigmoid)
                yield 0

        router_it = router_steps()

        def pump_router():
            try:
                next(router_it)
            except StopIteration:
                pass

        yg_r = yg.rearrange("(m p) d -> p m d", p=128)
        ys_r = ys.rearrange("(m p) d -> p m d", p=128)

        def mm2_group(mt, ht_col0, w2t, seed, out_r, scale, gname,
                      interleave_router=False, last=False):
            """One mm2 m-tile: 2 n-halves x full k loop, bf16 eviction."""
            pos = []
            for n in range(2):
                po = psp.tile([128, 512], F32, tag="ps", name=f"po{gname}_{n}")
                pos.append(po)
                if seed is not None:
                    nc.tensor.matmul(
                        po, onesm_b[:], seed[:, n * 512 : (n + 1) * 512],
                        start=True, stop=False)
            for k in range(KH):
                for n in range(2):
                    nc.tensor.matmul(
                        pos[n],
                        ht[:, k, ht_col0 : ht_col0 + 128],
                        w2t[:, k, n * 512 : (n + 1) * 512],
                        start=(k == 0 and seed is None),
                        stop=(k == KH - 1))
                if interleave_router:
                    pump_router()
            for n in range(2):
                og = ost.tile([128, 512], BF16, tag="og", name=f"og{gname}_{n}")
                # split the two n-half evictions across DVE and ACT so each
                # group's drain runs on two engines in parallel
                if scale:
                    if n == 0:
                        nc.vector.tensor_scalar(
                            og, pos[n][:], wv[mt][:], None, op0=AluOpType.mult)
                    else:
                        nc.scalar.activation(
                            og, pos[n][:], AF.Copy, scale=wv[mt][:])
                else:
                    if n == 0:
                        nc.vector.tensor_copy(og, pos[n][:])
                    else:
                        nc.scalar.activation(og, pos[n][:], AF.Copy)
                # for the final group issue the two stores on different
                # queues so the terminal drain is one store, not two
                oeng = nc.scalar if (last and n == 1) else nc.sync
                oeng.dma_start(
                    out=out_r[:, mt, n * 512 : (n + 1) * 512], in_=og)

        # ---- expert mm2 (w2 resident; router pumped between k-steps) ----
        eb2seed = b2row if has_b2 else None
        sb2seed = sb2row if has_b2 else None
        for mt in range(MTE):
            mm2_group(mt, mt * 128, w2_sb, eb2seed, yg_r, True, f"e{mt}",
                      interleave_router=True)

        # ---- shared mm2 (sw2 resident) ----
        for mt in range(SME):
            mm2_group(mt, cap + mt * 128, sw2_sb, sb2seed, ys_r, False, f"s{mt}",
                      last=(mt == SME - 1))

    nc.compile()
    return nc


_programs: dict = {}
LAST_RESULTS = None


def _get_program(key):
    if key not in _programs:
        _programs[key] = build_program(*key)
    return _programs[key]


def kernel(x, router_w, router_b, sw1, sb1, sw2, sb2, ew1, eb1, ew2, eb2):
    x = np.asarray(x, dtype=np.float32)
    flat = np.ascontiguousarray(x.reshape(T, D))
    rw_in = np.ascontiguousarray(np.asarray(router_w, np.float32))
    rb_in = np.asarray(router_b, np.float32).reshape(1, E)

    # Integer dispatch decisions only: which two experts each token visits.
    logits = flat @ rw_in + rb_in
    top1 = np.argmax(logits, axis=1)
    l2m = np.array(logits)
    l2m[np.arange(T), top1] = -np.inf
    top2 = np.argmax(l2m, axis=1)
    idxs = [np.flatnonzero((top1 == e) | (top2 == e)) for e in range(E)]

    # Assign each token a "shared host" core among its top-2 experts with
    # exactly SH tokens per core, so the shared-expert input is a subset of
    # the core's gathered tokens (saves a separate xsb shard + its DMA).
    # Integer decisions only.  Fallback: excess tokens append to any
    # under-loaded core as pseudo-gathered rows (their yg is discarded).
    host = top1.copy()
    load = np.bincount(host, minlength=E)
    for _ in range(4 * E):
        over = np.flatnonzero(load > SH)
        if len(over) == 0:
            break
        moved = False
        for c in over:
            for t in np.flatnonzero(host == c):
                alt = top2[t] if top1[t] == c else top1[t]
                if load[alt] < SH:
                    host[t] = alt
                    load[c] -= 1
                    load[alt] += 1
                    moved = True
                    if load[c] <= SH:
                        break
        if not moved:
            break
    for c in np.flatnonzero(load > SH):  # last resort: append anywhere
        for t in np.flatnonzero(host == c):
            if load[c] <= SH:
                break
            u = int(np.argmin(load))
            host[t] = u
            load[c] -= 1
            load[u] += 1

    gathers = []
    for c in range(NCORES):
        sh_c = np.flatnonzero(host == c)
        rest = idxs[c][host[idxs[c]] != c]
        gathers.append(np.concatenate([sh_c, rest]))
    maxn = max(len(g) for g in gathers)
    cap = max(DEFAULT_CAP, ((maxn + 127) // 128) * 128)
    MTE = cap // 128

    has_b1 = bool(np.any(sb1)) or bool(np.any(eb1))
    has_b2 = bool(np.any(sb2)) or bool(np.any(eb2))
    has_rb = bool(np.any(router_b))
    nc = _get_program((cap, has_b1, has_b2, has_rb))

    # rw / dsel pre-permuted to [128, k*E] (partition-major) wide-line DMAs
    rw_perm = np.ascontiguousarray(
        rw_in.reshape(KD, 128, E).transpose(1, 0, 2).reshape(128, KD * E)
    ).astype(BF)

    sw1b = np.ascontiguousarray(np.asarray(sw1)).astype(BF)
    sw2b = np.ascontiguousarray(np.asarray(sw2)).astype(BF)
    sb1b = np.asarray(sb1).reshape(1, H).astype(BF)
    sb2b = np.asarray(sb2).reshape(1, D).astype(BF)
    ew1a = np.asarray(ew1)
    ew2a = np.asarray(ew2)
    eb1a = np.asarray(eb1)
    eb2a = np.asarray(eb2)

    in_maps = []
    real_rows = []
    for c in range(NCORES):
        g = gathers[c]
        n = len(g)
        xg = np.zeros((cap, D), np.float32)
        xg[:n] = flat[g]
        xgt_b = np.ascontiguousarray(xg.T).astype(BF)
        # dsel: +1 at own expert, -1 at the partner expert (integer
        # dispatch info); zero rows for padding and for appended
        # shared-only pseudo rows -> their yg contribution is discarded.
        is_real = (top1[g] == c) | (top2[g] == c)
        rr = np.flatnonzero(is_real)
        real_rows.append(rr)
        other = np.where(top1[g[rr]] == c, top2[g[rr]], top1[g[rr]])
        dsel_c = np.zeros((cap, E), np.float32)
        dsel_c[rr, c] = 1.0
        dsel_c[rr, other] = -1.0
        dsel_perm = np.ascontiguousarray(
            dsel_c.reshape(MTE, 128, E).transpose(1, 0, 2).reshape(128, MTE * E))
        in_maps.append({
            "xgt_b": xgt_b,
            "xs0": np.ascontiguousarray(flat[g[:SH]].T).astype(BF),
            "rw": rw_perm,
            "dsel": dsel_perm,
            "rb": rb_in,
            "w1": np.ascontiguousarray(ew1a[c]).astype(BF),
            "w2": np.ascontiguousarray(ew2a[c]).astype(BF),
            "sw1": sw1b,
            "sw2": sw2b,
            "b1": np.asarray(eb1a[c]).reshape(1, H).astype(BF),
            "b2": np.asarray(eb2a[c]).reshape(1, D).astype(BF),
            "sb1": sb1b,
            "sb2": sb2b,
        })

    res = None
    for attempt in range(5):
        try:
            res = run_bass_kernel_spmd(nc, in_maps, core_ids=list(range(NCORES)))
            break
        except Exception:
            if attempt == 4:
                raise
            import time as _time
            _time.sleep(25)  # wedged-device windows recover after ~1-2 min
    global LAST_RESULTS
    LAST_RESULTS = res

    y = np.zeros((T, D), np.float32)
    for c in range(NCORES):
        g = gathers[c]
        y[g[:SH]] += res.results[c]["ys"].astype(np.float32)
        rr = real_rows[c]
        y[g[rr]] += res.results[c]["yg"][rr].astype(np.float32)
    return y.reshape(B, S, D)
